# revision 18
# baseline (speedup 1.0000x reference)
"""AttnVLAD layer on 8 Trainium2 NeuronCores.

Data-parallel over batch: b=32 samples -> 4 per core. Params
(centers/alpha/cluster_weights) replicated. Per sample:
  mm1: scoreT[n,128] = xh_block^T @ (qh|ql)  (x stationary, packed
       2-term fp16 q-split rhs, N=128 per matmul)
  z[n,K] = qh-half + ql-half (PSUM free-dim halves add)
  softmax over K, batched across all 32 n-chunks in one chain
  mm2: descT[K,D] = probs^T @ xT16 (prob stationary, fp16, slid one
       sample later to overlap with next sample's mm1)
  epilogue: denom-normalize, subtract centers, intra-L2, cluster
  weights, global L2 -> out[d*K]
"""
import numpy as np

B, D, N, K = 32, 512, 4096, 64
NCORES = 8
SPC = B // NCORES          # samples per core
DCH = D // 128             # 4 d-chunks
NCH = N // 128             # 32 n-chunks
GCH = 4                    # n-chunks per score PSUM group (1 bank)
GRP = NCH // GCH           # 8 groups per sample

_COMPILED = {}


def _build():
    import concourse.bass as bass
    import concourse.bacc as bacc
    import concourse.tile as tile
    import concourse.mybir as mybir

    f32 = mybir.dt.float32
    f16 = mybir.dt.float16
    AF = mybir.ActivationFunctionType
    OP = mybir.AluOpType
    AX = mybir.AxisListType

    nc = bacc.Bacc("TRN2", target_bir_lowering=False, debug=False)
    xc_dram = nc.dram_tensor("xc", [SPC, D, N], f32, kind="ExternalInput")
    c_dram = nc.dram_tensor("centers", [D, K], f32, kind="ExternalInput")
    alpha_dram = nc.dram_tensor("alpha", [1, 1], f32, kind="ExternalInput")
    cw_dram = nc.dram_tensor("cw", [K, 1], f32, kind="ExternalInput")
    id_dram = nc.dram_tensor("ident", [128, 128], f32, kind="ExternalInput")
    out_dram = nc.dram_tensor("out", [SPC, D * K], f32, kind="ExternalOutput")

    with tile.TileContext(nc) as tc:
        with (
            tc.tile_pool(name="const", bufs=1) as const,
            tc.tile_pool(name="xpool", bufs=2) as xpool,
            tc.tile_pool(name="xhp", bufs=7) as xhp,
            tc.tile_pool(name="xTp", bufs=2) as xTp,
            tc.tile_pool(name="smp", bufs=2) as smp,
            tc.tile_pool(name="probp", bufs=3) as probp,
            tc.tile_pool(name="epp", bufs=1) as epp,
            tc.tile_pool(name="ps_sc", bufs=2, space="PSUM") as ps_sc,
            tc.tile_pool(name="ps_d", bufs=2, space="PSUM") as ps_d,
            tc.tile_pool(name="ps_n", bufs=1, space="PSUM") as ps_n,
            tc.tile_pool(name="ps_m", bufs=1, space="PSUM") as ps_m,
        ):
            # ---------- one-time prep ----------
            ident = const.tile([128, 128], f32, tag="ident")
            nc.sync.dma_start(ident[:], id_dram[:])
            c_sb = const.tile([128, DCH, K], f32, tag="c_sb")
            nc.sync.dma_start(
                c_sb[:], c_dram[:].rearrange("(c p) k -> p c k", p=128))
            alpha_sb = const.tile([1, 1], f32, tag="alpha_sb")
            nc.sync.dma_start(alpha_sb[:], alpha_dram[:])
            cw_sb = const.tile([K, 1], f32, tag="cw_sb")
            nc.sync.dma_start(cw_sb[:], cw_dram[:])
            ones16 = const.tile([128, 1], f16, tag="ones16")
            nc.gpsimd.memset(ones16[:], 1.0)
            onesc = const.tile([128, 1], f32, tag="onesc")
            nc.gpsimd.memset(onesc[:], 1.0)
            onesr = const.tile([1, 128], f32, tag="onesr")
            nc.gpsimd.memset(onesr[:], 1.0)

            # q_s = centers * (alpha / max(||c||_d, 1e-12)), fp32 [128, DCH, K]
            sq = const.tile([128, DCH, K], f32, tag="sq")
            nc.vector.tensor_mul(sq[:], c_sb[:], c_sb[:])
            ssum = ps_m.tile([1, K], f32, tag="m")
            for dc in range(DCH):
                nc.tensor.matmul(ssum[:], onesc[:], sq[:, dc, :],
                                 start=(dc == 0), stop=(dc == DCH - 1))
            cnorm = const.tile([1, K], f32, tag="cnorm")
            nc.scalar.activation(cnorm[:], ssum[:], AF.Sqrt)
            nc.vector.tensor_scalar_max(cnorm[:], cnorm[:], 1e-12)
            cscale = const.tile([1, K], f32, tag="cscale")
            nc.vector.reciprocal(cscale[:], cnorm[:])
            nc.vector.tensor_scalar_mul(cscale[:], cscale[:], alpha_sb[:])
            scale_rep = ps_m.tile([128, K], f32, tag="m")
            nc.tensor.matmul(scale_rep[:], onesr[:], cscale[:],
                             start=True, stop=True)
            q_s = const.tile([128, DCH, K], f32, tag="q_s")
            for dc in range(DCH):
                nc.vector.tensor_mul(q_s[:, dc, :], c_sb[:, dc, :],
                                     scale_rep[:])
            # packed fp16 split of q_s: qpack[:, dc, 0:K] = qh = f16(q_s),
            # qpack[:, dc, K:2K] = ql = f16(q_s - qh)
            qpack = const.tile([128, DCH, 2 * K], f16, tag="qpack")
            for dc in range(DCH):
                nc.vector.tensor_copy(qpack[:, dc, 0:K], q_s[:, dc, :])
            qh32 = const.tile([128, DCH, K], f32, tag="qh32")
            for dc in range(DCH):
                nc.vector.tensor_copy(qh32[:, dc, :], qpack[:, dc, 0:K])
                nc.vector.tensor_sub(qpack[:, dc, K:2 * K], q_s[:, dc, :],
                                     qh32[:, dc, :])

            # centersT [K, D] for the epilogue subtract
            cT = const.tile([K, D], f32, tag="cT")
            for dc in range(DCH):
                tp = ps_m.tile([K, 128], f32, tag="m")
                nc.tensor.transpose(tp[:], c_sb[:, dc, :], ident[:])
                nc.scalar.copy(cT[:, dc * 128:(dc + 1) * 128], tp[:])

            # ---------- per-sample pipeline ----------
            def load_chunk(s, dc):
                x32 = xpool.tile([128, N], f32, tag="x32",
                                 name=f"x32_{s}_{dc}")
                # loads go through the Activation HWDGE ring so they never
                # queue behind the xbar transposes on the Sync ring
                nc.scalar.dma_start(x32[:], xc_dram[s, dc * 128:(dc + 1) * 128, :])
                xh = xhp.tile([128, N], f16, tag="xh", name=f"xh_{s}_{dc}")
                nc.vector.tensor_copy(xh[:], x32[:])
                return xh

            # prefetch sample 0
            xcur = [load_chunk(0, dc) for dc in range(DCH)]
            # deferred work from the previous sample:
            #   pend_mm2[g] emits mm2 chunks for group g, pend_ep emits epilogue
            pend_mm2 = None
            pend_ep = None

            HCH = NCH // 2  # n-chunks per half-sample softmax batch

            for s in range(SPC):
                # transpose xh -> xT16 [p, dc, j, 128] for mm2
                xT16 = xTp.tile([128, DCH, NCH, 128], f16, tag="xT16")
                for dc in range(DCH):
                    nc.sync.dma_start_transpose(xT16[:, dc, :, :],
                                                xcur[dc][:])

                xnext = [None] * DCH
                probs_h = [None, None]

                for h in range(2):
                    # mm1 half: score[n,K] accumulated in PSUM; qh and ql
                    # terms go to the SAME slice (the PE does the 2-term add)
                    sc = ps_sc.tile([128, HCH, K], f32, tag="scoreT",
                                    name=f"sc_{s}_{h}")
                    for dc in range(DCH):
                        for c in range(HCH):
                            j = h * HCH + c
                            sl = slice(j * 128, (j + 1) * 128)
                            last = (dc == DCH - 1 and c == HCH - 1)
                            # start=True must hit the first write of EACH
                            # psum bank (8 chunks of 64 fp32 per 2KB bank)
                            bank_first = (dc == 0 and c % 8 == 0)
                            nc.tensor.matmul(
                                sc[:, c, :], xcur[dc][:, sl],
                                qpack[:, dc, 0:K],
                                start=bank_first, stop=False,
                                skip_group_check=(not (dc == 0 and c == 0)))
                            nc.tensor.matmul(
                                sc[:, c, :], xcur[dc][:, sl],
                                qpack[:, dc, K:2 * K],
                                start=False, stop=last,
                                skip_group_check=True)
                        # previous sample's mm2, 4 chunks per d-pass
                        if pend_mm2 is not None:
                            pend_mm2(h * DCH + dc)
                        # staggered prefetch of the next sample (last chunk
                        # loads after the loop to cap xhp at 7 live bufs)
                        if s + 1 < SPC and h == 0 and dc < DCH - 1:
                            xnext[dc] = load_chunk(s + 1, dc)

                    # ---- softmax over K, batched over 16 chunks ----
                    negmax = smp.tile([128, HCH], f32, tag="negmax",
                                      name=f"nm{s}_{h}")
                    nc.vector.reduce_max(negmax[:].unsqueeze(2), sc[:],
                                         axis=AX.X, negate=True)
                    zc = smp.tile([128, HCH, K], f32, tag="zc",
                                  name=f"zc{s}_{h}")
                    nc.vector.tensor_add(
                        zc[:], sc[:],
                        negmax[:].unsqueeze(2).broadcast_to([128, HCH, K]))
                    e16 = smp.tile([128, HCH, K], f16, tag="e16",
                                   name=f"e{s}_{h}")
                    nc.scalar.activation(e16[:].rearrange("p a b -> p (a b)"),
                                         zc[:].rearrange("p a b -> p (a b)"),
                                         AF.Exp)
                    rs = smp.tile([128, HCH], f32, tag="rs", name=f"rs{s}_{h}")
                    nc.vector.reduce_sum(rs[:].unsqueeze(2), e16[:],
                                         axis=AX.X)
                    rr = smp.tile([128, HCH], f32, tag="rr", name=f"rr{s}_{h}")
                    nc.vector.reciprocal(rr[:], rs[:])
                    probs = probp.tile([128, HCH, K], f16, tag="prob",
                                       name=f"pr{s}_{h}")
                    nc.vector.tensor_mul(
                        probs[:], e16[:],
                        rr[:].unsqueeze(2).broadcast_to([128, HCH, K]))
                    probs_h[h] = probs

                # epilogue of s-1 only after ALL of mm2(s-1) is emitted
                if pend_ep is not None:
                    pend_ep()
                    pend_ep = None

                if s + 1 < SPC:
                    xnext[DCH - 1] = load_chunk(s + 1, DCH - 1)
                    xcur = xnext

                descT = ps_d.tile([K, D], f32, tag="descT", name=f"dT{s}")
                denom = ps_n.tile([K, 1], f32, tag="denom", name=f"dn{s}")

                def make_mm2(s, descT, denom, probs_h, xT16):
                    def run(g):
                        for c in range(GCH):
                            j = g * GCH + c
                            pch = probs_h[j // HCH][:, j % HCH, :]
                            nc.tensor.matmul(descT[:], pch,
                                             xT16[:, :, j, :],
                                             start=(j == 0),
                                             stop=(j == NCH - 1))
                            nc.tensor.matmul(denom[:], pch,
                                             ones16[:],
                                             start=(j == 0),
                                             stop=(j == NCH - 1))
                    return run

                def make_epilogue(s, descT, denom):
                    def run():
                        # ------- epilogue (descT [K, D] layout) -------
                        rdenom = epp.tile([K, 1], f32, tag="rdenom",
                                          name=f"rd{s}")
                        nc.vector.tensor_scalar_max(rdenom[:], denom[:], 1e-6)
                        nc.vector.reciprocal(rdenom[:], rdenom[:])
                        desc_c = epp.tile([K, D], f32, tag="desc_c",
                                          name=f"dcc{s}")
                        nc.vector.scalar_tensor_tensor(
                            desc_c[:], in0=descT[:], scalar=rdenom[:],
                            in1=cT[:], op0=OP.mult, op1=OP.subtract)
                        sqe = epp.tile([K, D], f32, tag="sqe", name=f"sq{s}")
                        nc.vector.tensor_mul(sqe[:], desc_c[:], desc_c[:])
                        ss = epp.tile([K, 1], f32, tag="ss", name=f"ss{s}")
                        nc.vector.reduce_sum(ss[:], sqe[:], axis=AX.X)
                        intra = epp.tile([K, 1], f32, tag="intra",
                                         name=f"in{s}")
                        nc.scalar.activation(intra[:], ss[:], AF.Sqrt)
                        nc.vector.tensor_scalar_max(intra[:], intra[:], 1e-12)
                        rintra = epp.tile([K, 1], f32, tag="rintra",
                                          name=f"ri{s}")
                        nc.vector.reciprocal(rintra[:], intra[:])
                        cwr = epp.tile([K, 1], f32, tag="cwr", name=f"cw{s}")
                        nc.vector.tensor_mul(cwr[:], cw_sb[:], rintra[:])
                        t1 = epp.tile([K, 1], f32, tag="t1", name=f"t1{s}")
                        nc.vector.tensor_mul(t1[:], ss[:], cwr[:])
                        nc.vector.tensor_mul(t1[:], t1[:], cwr[:])
                        tot = ps_m.tile([1, 1], f32, tag="m", name=f"to{s}")
                        nc.tensor.matmul(tot[:], t1[:], onesc[:K, :],
                                         start=True, stop=True)
                        fin = epp.tile([1, 1], f32, tag="fin", name=f"fi{s}")
                        nc.scalar.activation(fin[:], tot[:], AF.Sqrt)
                        nc.vector.tensor_scalar_max(fin[:], fin[:], 1e-12)
                        nc.vector.reciprocal(fin[:], fin[:])
                        finrep = ps_m.tile([K, 1], f32, tag="m",
                                           name=f"fr{s}")
                        nc.tensor.matmul(finrep[:], onesr[:, :K], fin[:],
                                         start=True, stop=True)
                        sfin = epp.tile([K, 1], f32, tag="sfin",
                                        name=f"sf{s}")
                        nc.vector.tensor_mul(sfin[:], cwr[:], finrep[:])
                        outT = epp.tile([K, D], f32, tag="outT",
                                        name=f"oT{s}")
                        nc.vector.tensor_mul(outT[:], desc_c[:],
                                             sfin[:].broadcast_to([K, D]))
                        # transpose [K, D] -> [D, K] blocks, DMA out
                        for dc in range(DCH):
                            tp = ps_m.tile([128, K], f32, tag="m",
                                           name=f"tp{s}_{dc}")
                            nc.tensor.transpose(
                                tp[:], outT[:, dc * 128:(dc + 1) * 128],
                                ident[:K, :K])
                            ot = epp.tile([128, K], f32, tag="ot",
                                          name=f"ot{s}_{dc}")
                            nc.scalar.copy(ot[:], tp[:])
                            nc.scalar.dma_start(
                                out_dram[s, dc * 128 * K:(dc + 1) * 128 * K]
                                .rearrange("(p k) -> p k", k=K),
                                ot[:])
                    return run

                pend_mm2 = make_mm2(s, descT, denom, probs_h, xT16)
                pend_ep = make_epilogue(s, descT, denom)
                if s == SPC - 1:
                    for g in range(GRP):
                        pend_mm2(g)
                    pend_ep()
                    pend_mm2 = None
                    pend_ep = None

    nc.compile()
    return nc


def kernel(x, centers, alpha, cluster_weights):
    import concourse.bass_utils as bass_utils

    if "nc" not in _COMPILED:
        _COMPILED["nc"] = _build()
    nc = _COMPILED["nc"]

    x = np.ascontiguousarray(np.asarray(x, dtype=np.float32))
    c = np.asarray(centers, dtype=np.float32).reshape(D, K)
    a = np.asarray(alpha, dtype=np.float32).reshape(1, 1)
    cw = np.asarray(cluster_weights, dtype=np.float32).reshape(K, 1)
    ident = np.eye(128, dtype=np.float32)

    in_maps = []
    for core in range(NCORES):
        in_maps.append({
            "xc": x[core * SPC:(core + 1) * SPC],
            "centers": c,
            "alpha": a,
            "cw": cw,
            "ident": ident,
        })
    res = bass_utils.run_bass_kernel_spmd(nc, in_maps,
                                          core_ids=list(range(NCORES)))
    out = np.concatenate([res.results[i]["out"] for i in range(NCORES)],
                         axis=0)
    return out.astype(np.float32)
